# revision 22
# baseline (speedup 1.0000x reference)
"""AttentionBlock Trainium2 kernel (fp8-DoubleRow edition).

Reference computation (B=16, C=512, H=W=32, n_heads=4, d_k=128):
    xs   = x.reshape(B,C,S).T            # [B, S, C],  S = 1024
    qkv  = xs @ w_proj.T + b_proj        # [B, S, 1536]
    S_   = einsum('bihd,bjhd->bijh', q, k) * d_k**-0.5
    attn = softmax(S_, axis=1)           # over the QUERY axis i (source quirk)
    res  = einsum('bijh,bjhd->bihd', attn, v)
    out  = res @ w_out.T + b_out + xs    # residual
    return out.T.reshape(B, C, H, W)

Strategy: data-parallel over batch, 2 batches per core on 8 cores. Transposed
layouts avoid on-device transposes.

Precision: the output is dominated by the fp32 residual xs (+b_out); the
attention path contributes only ~8% of the output magnitude (std 0.08 vs
1.0), so it tolerates fp8. The QKV/output projections and the AV contraction
run as fp8e4 DoubleRow matmuls (K=256 per pass: measured ~207-244ns vs 270ns
for a bf16 K=128 matmul). Scores stay bf16 (contraction is only d_k=128, so
DoubleRow can't help, and it keeps exp() inputs accurate). The softmax runs
as exp on ACT with fp8 output + free accum_out row sums; the normalizer is
folded into per-(head,j-tile) fp8 V rows (v_sc = 64*v/Z) on the DVE — only
tiny ops sit in the PE->ACT->DVE->PE chain (bulk DVE ops there measurably
stall the strict-FIFO queue). fp8 range handling:
  wqk *= 16   -> qk_sb holds 16*(q|k) in bf16; exp scale absorbs the 256
  wv  *= 64   -> v_sb holds 64*v (bf16); v_sc = 64*v/Z is fp8-normal
  wo  *= 16   -> out-proj psum = 1024*(res@wo.T); epilogue applies 1/1024
  exp bias=-2 (softmax shift-invariant) keeps e^s in [~0.007, ~40] for fp8
The residual + b_out add is one fp32 scalar_tensor_tensor on DVE against a
precomputed xb = x + b_out.

Scheduling: the softmax (ACT) is the second-busiest engine after PE, so all
PE-only projection work is interleaved into the ACT-bound attention phases:
next-head QK projections after each head, the NEXT batch's QK01+V projection
during the last head, and batch 0's output projection inside batch 1's head
phases. PSUM accumulation is emitted bank-interleaved (A,B,A,B) because
back-to-back same-bank accumulating matmuls measure ~+60ns each.
"""
import sys

for _p in (
    "/opt/trn_rl_repo",
    "/root/.axon_site",
    "/root/.axon_site/_ro/trn_rl_repo",
    "/root/.axon_site/_ro/pypackages",
):
    if _p not in sys.path:
        sys.path.append(_p)

import numpy as np

B = 16
C = 512
S = 1024  # H*W
NH = 4
DK = 128
F = NH * DK  # 512
NCORES = 8
BL = B // NCORES  # batches per core
KT = C // 128  # 4  contraction tiles over channels
ST = S // 128  # 8  seq tiles
NT = S // 512  # 2  free-dim chunks of 512
SCALE = float(DK) ** -0.5
WQK_SCALE = 16.0  # host pre-scale on w_qkT/b_qk -> scores psum = 256*s
WV_SCALE = 64.0  # host pre-scale on w_vT/b_v -> racc = 64*res
WO_SCALE = 16.0  # host pre-scale on w_outT -> out psum = 1024*out_attn
ESHIFT = -2.0  # exp(s*scale + ESHIFT): softmax-invariant fp8 range shift

_CACHE: dict = {}


def _build(repeat=1, unroll=1):
    """Build the kernel. repeat>1 wraps the per-call workload in an on-device
    For_i loop — used only for timing (amortizes the ~10ms axon dispatch).
    unroll>1 emits the workload N times inline (cost-model analysis only)."""
    import contextlib

    import concourse.tile as tile
    from concourse import bacc, mybir

    F32 = mybir.dt.float32
    F32R = mybir.dt.float32r
    BF16 = mybir.dt.bfloat16
    FP8 = mybir.dt.float8e4
    EXP = mybir.ActivationFunctionType.Exp

    nc = bacc.Bacc("TRN2", debug=False)
    x_d = nc.dram_tensor("x", [BL, C, S], F32, kind="ExternalInput").ap()
    wqk_d = nc.dram_tensor("w_qkT", [C, 2 * F], F32, kind="ExternalInput").ap()
    wv_d = nc.dram_tensor("w_vT", [C, F], F32, kind="ExternalInput").ap()
    wo_d = nc.dram_tensor("w_outT", [F, C], F32, kind="ExternalInput").ap()
    bias_d = nc.dram_tensor("bias", [128, 2 * NH + 2 * F + KT + 1], F32, kind="ExternalInput").ap()
    out_d = nc.dram_tensor("out", [BL, C, S], F32, kind="ExternalOutput").ap()

    wqk_r = wqk_d.rearrange("(k p) m -> p k m", p=128)
    wv_r = wv_d.rearrange("(k p) m -> p k m", p=128)
    wo_r = wo_d.rearrange("(k p) m -> p k m", p=128)

    with tile.TileContext(nc) as tc:
        with (
            tc.tile_pool(name="const", bufs=1) as constp,
            tc.tile_pool(name="stage", bufs=1) as stagep,
            tc.tile_pool(name="xp", bufs=2) as xp,
            tc.tile_pool(name="qkp", bufs=2) as qkp,
            tc.tile_pool(name="vp", bufs=2) as vp,
            tc.tile_pool(name="ep", bufs=3) as ep,
            tc.tile_pool(name="etp", bufs=3) as etp,
            tc.tile_pool(name="rp", bufs=2) as rp,
            tc.tile_pool(name="op", bufs=2) as op,
            tc.tile_pool(name="small", bufs=16) as smallp,
            tc.tile_pool(name="vs", bufs=8) as vsp,
            # psum: pp = [128,512]x2 shared by qk/v/out projections;
            # ps = [128,1024]x2 for score tiles; pr = [128,512]x2 for the
            # per-head AV accumulators. 2+4+2 = 8 banks.
            tc.tile_pool(name="pp", bufs=2, space="PSUM") as pp,
            tc.tile_pool(name="ps", bufs=2, space="PSUM") as ps,
            tc.tile_pool(name="pr", bufs=2, space="PSUM") as pr,
        ):
            # ---- constants: load fp32, convert once to fp8 ----
            wqk8_sb = constp.tile([128, KT, 2 * F], FP8)
            wv8_sb = constp.tile([128, KT, F], FP8)
            wo8_sb = constp.tile([128, KT, C], FP8)
            bias_sb = constp.tile([128, 2 * NH + 2 * F + KT + 1], F32)
            x_sbs = [xp.tile([128, KT, S], F32, name=f"x{b}", tag="x") for b in range(BL)]
            x8_sbs = [xp.tile([128, KT, S], FP8, name=f"x8{b}", tag="x8") for b in range(BL)]
            # xb = x + b_out (residual + out bias pre-added once; the out-proj
            # epilogue is then a single scalar_tensor_tensor per half)
            xb_sbs = [xp.tile([128, KT, S], F32, name=f"xb{b}", tag="xb") for b in range(BL)]

            wqk_st = stagep.tile([128, KT, 2 * F], F32, name="wst", tag="wst")
            for k in range(KT):
                nc.sync.dma_start(out=x_sbs[0][:, k, :], in_=x_d[0, bass_ts(k, 128), :])
                nc.sync.dma_start(out=wqk_st[:, k, :], in_=wqk_r[:, k, :])
            nc.vector.tensor_copy(x8_sbs[0], x_sbs[0])
            nc.gpsimd.tensor_copy(wqk8_sb, wqk_st)
            wv_st = stagep.tile([128, KT, F], F32, name="wst", tag="wst")
            nc.sync.dma_start(out=wv_st, in_=wv_r)
            nc.gpsimd.tensor_copy(wv8_sb, wv_st)
            nc.sync.dma_start(out=bias_sb, in_=bias_d)
            for b in range(1, BL):
                for k in range(KT):
                    nc.sync.dma_start(out=x_sbs[b][:, k, :], in_=x_d[b, bass_ts(k, 128), :])
                nc.vector.tensor_copy(x8_sbs[b], x_sbs[b])
            wo_st = stagep.tile([128, KT, C], F32, name="wst", tag="wst")
            nc.sync.dma_start(out=wo_st, in_=wo_r)
            nc.gpsimd.tensor_copy(wo8_sb, wo_st)
            for b in range(BL):
                for k in range(KT):
                    nc.gpsimd.tensor_scalar_add(
                        xb_sbs[b][:, k, :],
                        x_sbs[b][:, k, :],
                        bias_sb[:, 2 * NH + 2 * F + k : 2 * NH + 2 * F + k + 1],
                    )
            b_qk = bias_sb[:, 0 : 2 * NH]  # per-partition bias per qk f-tile (x16)
            b_v2 = bias_sb[:, 2 * NH : 2 * NH + 2 * F]  # v bias doubled (x64)
            b_esh = bias_sb[:, 2 * NH + 2 * F + KT :]  # ESHIFT constant column

            env = dict(
                nc=nc, qkp=qkp, vp=vp, ep=ep, etp=etp, rp=rp, op=op, smallp=smallp,
                vsp=vsp, pp=pp, ps=ps, pr=pr, wqk8_sb=wqk8_sb, wv8_sb=wv8_sb,
                wo8_sb=wo8_sb, b_qk=b_qk, b_v2=b_v2, b_esh=b_esh, out_d=out_d,
                x8_sbs=x8_sbs, xb_sbs=xb_sbs, F32=F32, F32R=F32R, BF16=BF16, FP8=FP8,
                EXP=EXP, mybir=mybir,
            )
            # software-pipeline prologue: batch 0's QK01 + V projections
            pend = {0: _proj_phase(env, 0)}

            rep_ctx = (
                tc.For_i(0, repeat, 1) if repeat > 1 else contextlib.nullcontext()
            )
            with rep_ctx:
                for _u in range(unroll):
                    _batches(env, pend)

    nc.compile()
    return nc


def _proj_phase(env, b):
    """QK f-tiles 0,1 + full V projection for batch b. Returns (qk_sb, v_sb)."""
    nc = env["nc"]
    F32, BF16 = env["F32"], env["BF16"]
    x8 = env["x8_sbs"][b]
    pp, b_qk, b_v2 = env["pp"], env["b_qk"], env["b_v2"]
    wv8_sb = env["wv8_sb"]

    qk_sb = env["qkp"].tile([128, 2 * NH, S], env["BF16"], name=f"qk{b}", tag="qk")
    _qk_proj(env, x8, qk_sb, 0, 1)
    v_sb = env["vp"].tile([128, ST, F], env["BF16"], name=f"v{b}", tag="v")
    for stp in range(ST // 2):
        st, st2 = 2 * stp, 2 * stp + 1
        acc_a = pp.tile([128, 512], F32, name="va", tag="pp")
        acc_b = pp.tile([128, 512], F32, name="vb", tag="pp")
        for k2 in range(2):
            nc.tensor.matmul(
                acc_a,
                x8[:, 2 * k2 : 2 * k2 + 2, bass_ts(st, 128)],
                wv8_sb[:, 2 * k2 : 2 * k2 + 2, :],
                start=(k2 == 0),
                stop=(k2 == 1),
                perf_mode=_dr(),
            )
            nc.tensor.matmul(
                acc_b,
                x8[:, 2 * k2 : 2 * k2 + 2, bass_ts(st2, 128)],
                wv8_sb[:, 2 * k2 : 2 * k2 + 2, :],
                start=(k2 == 0),
                stop=(k2 == 1),
                perf_mode=_dr(),
            )
        nc.vector.tensor_add(v_sb[:, st, :], acc_a, b_v2[:, 0:F])
        nc.vector.tensor_add(v_sb[:, st2, :], acc_b, b_v2[:, 0:F])
    return qk_sb, v_sb


def _qk_proj(env, x8, qk_sb, t, t2):
    # Q^T/K^T f-tiles t,t2: qk_sb[:, t, s] = 16*w_qkT[:, t].T @ x
    # Two DoubleRow k-blocks (channels 0:256, 256:512), A/B bank
    # interleaved so same-bank accumulating MMs are 2 apart.
    nc = env["nc"]
    F32 = env["F32"]
    pp, wqk8_sb, b_qk = env["pp"], env["wqk8_sb"], env["b_qk"]
    for n in range(NT):
        acc_a = pp.tile([128, 512], F32, name="qka", tag="pp")
        acc_b = pp.tile([128, 512], F32, name="qkb", tag="pp")
        for k2 in range(2):
            nc.tensor.matmul(
                acc_a,
                wqk8_sb[:, 2 * k2 : 2 * k2 + 2, bass_ts(t, 128)],
                x8[:, 2 * k2 : 2 * k2 + 2, bass_ts(n, 512)],
                start=(k2 == 0),
                stop=(k2 == 1),
                perf_mode=_dr(),
            )
            nc.tensor.matmul(
                acc_b,
                wqk8_sb[:, 2 * k2 : 2 * k2 + 2, bass_ts(t2, 128)],
                x8[:, 2 * k2 : 2 * k2 + 2, bass_ts(n, 512)],
                start=(k2 == 0),
                stop=(k2 == 1),
                perf_mode=_dr(),
            )
        nc.vector.tensor_scalar_add(
            qk_sb[:, t, bass_ts(n, 512)], acc_a, b_qk[:, t : t + 1]
        )
        nc.vector.tensor_scalar_add(
            qk_sb[:, t2, bass_ts(n, 512)], acc_b, b_qk[:, t2 : t2 + 1]
        )


def _out_proj_chunk(env, b, ct, resT_sb):
    """Output projection c-tile ct for batch b + fused epilogue + store."""
    nc = env["nc"]
    F32, mybir = env["F32"], env["mybir"]
    pp, wo8_sb, out_d = env["pp"], env["wo8_sb"], env["out_d"]
    xb = env["xb_sbs"][b]

    out_t = env["op"].tile([128, S], F32, name="ot", tag="ot")
    accs = [pp.tile([128, 512], F32, name=f"oa{n}", tag="pp") for n in range(NT)]
    for f2 in range(2):
        for n in range(NT):
            nc.tensor.matmul(
                accs[n],
                wo8_sb[:, 2 * f2 : 2 * f2 + 2, bass_ts(ct, 128)],
                resT_sb[:, 2 * f2 : 2 * f2 + 2, bass_ts(n, 512)],
                start=(f2 == 0),
                stop=(f2 == 1),
                perf_mode=_dr(),
            )
    # fused epilogue: out = acc/1024 + (x + b_out), then store per half.
    # (must be DVE: GPSIMD cannot read PSUM)
    for n in range(NT):
        nc.vector.scalar_tensor_tensor(
            out_t[:, bass_ts(n, 512)],
            accs[n],
            1.0 / (WV_SCALE * WO_SCALE),
            xb[:, ct, bass_ts(n, 512)],
            mybir.AluOpType.mult,
            mybir.AluOpType.add,
        )
        nc.sync.dma_start(
            out=out_d[b, bass_ts(ct, 128), bass_ts(n, 512)],
            in_=out_t[:, bass_ts(n, 512)],
        )


def _batches(env, pend):
    """One full workload pass (both batches), software-pipelined.

    Batch b's attention phases interleave: next-head QK tiles (h0-h2), the
    NEXT batch's QK01+V projection (h3), and for b=1 the PREVIOUS batch's
    output projection (one c-tile per head phase). Batch 1's own output
    projection runs at the end (the only PE-only stretch left).
    """
    nc = env["nc"]
    F32, FP8, EXP = env["F32"], env["FP8"], env["EXP"]
    ep, rp, smallp, vsp = env["ep"], env["rp"], env["smallp"], env["vsp"]
    ps, pr = env["ps"], env["pr"]
    b_esh = env["b_esh"]

    resT = {}
    for b in range(BL):
        qk_sb, v_sb = pend.pop(b)
        x8_next = env["x8_sbs"][(b + 1) % BL]

        resT_sb = rp.tile([128, NH, S], FP8, name=f"resT{b}", tag="resT")
        resT[b] = resT_sb
        for h in range(NH):
            racc = [pr.tile([128, 512], F32, name=f"racc{n}", tag="racc") for n in range(NT)]
            for jtp in range(ST // 2):
                e2 = ep.tile([128, 2, S], FP8, name="e2", tag="e2")
                v_sc2 = vsp.tile([128, 2, DK], FP8, name="vsc", tag="vsc")
                for half in range(2):
                    jt = 2 * jtp + half
                    ssum = smallp.tile([128, 2], F32, name="ssum", tag="ssum")
                    # scores S^T[j, i] for one j-tile: [128, 1024] PSUM
                    # (2 banks); one exp pass over both halves, softmax
                    # denominator via accum_out. psum holds 256*s.
                    sacc = ps.tile([128, S], F32, name="sacc", tag="sacc")
                    for n in range(NT):
                        nc.tensor.matmul(
                            sacc[:, bass_ts(n, 512)],
                            qk_sb[:, 2 * h + 1, bass_ts(jt, 128)],
                            qk_sb[:, 2 * h, bass_ts(n, 512)],
                            start=True,
                            stop=True,
                        )
                    nc.scalar.activation(
                        out=e2[:, half, :],
                        in_=sacc,
                        func=EXP,
                        scale=SCALE / (WQK_SCALE * WQK_SCALE),
                        bias=b_esh,
                        accum_out=ssum[:, 0:1],
                    )
                    nc.vector.reciprocal(ssum[:, 1:2], ssum[:, 0:1])
                    nc.vector.tensor_scalar_mul(
                        v_sc2[:, half, :],
                        v_sb[:, jt, bass_ts(h, DK)],
                        ssum[:, 1:2],
                    )
                for n in range(NT):
                    nc.tensor.matmul(
                        racc[n],
                        v_sc2,
                        e2[:, :, bass_ts(n, 512)],
                        start=(jtp == 0),
                        stop=(jtp == ST // 2 - 1),
                        perf_mode=_dr(),
                    )
            for n in range(NT):
                nc.vector.tensor_copy(
                    resT_sb[:, h, bass_ts(n, 512)], racc[n]
                )
            # PE-only filler for this ACT-bound phase:
            if h + 1 < NH:
                _qk_proj(env, env["x8_sbs"][b], qk_sb, 2 * h + 2, 2 * h + 3)
            else:
                # last head: project the next batch (next iteration's b=0
                # when b is the last batch — recomputed there, harmless)
                pend[(b + 1) % BL] = _proj_phase(env, (b + 1) % BL)
            if b == BL - 1:
                # batch 0's output projection, one c-tile per head phase
                _out_proj_chunk(env, 0, h, resT[0])

    # ---- batch 1's output projection (tail) ----
    for ct in range(KT):
        _out_proj_chunk(env, BL - 1, ct, resT[BL - 1])


def _dr():
    from concourse import mybir

    return mybir.MatmulPerfMode.DoubleRow


def bass_ts(i, size):
    import concourse.bass as bass

    return bass.ts(i, size)


def _prep_inputs(x, w_proj, b_proj, w_out, b_out):
    """Host-side reshaping into the layouts the kernel expects."""
    x_f = np.ascontiguousarray(x.reshape(B, C, S), dtype=np.float32)
    wT = np.asarray(w_proj, dtype=np.float32).T  # [C, 3*F], f = h*384 + j
    w_qkT = WQK_SCALE * np.concatenate(
        [wT[:, h * 384 : h * 384 + 256] for h in range(NH)], axis=1
    )  # [C, 2F]; col tile t=2h -> q_h, t=2h+1 -> k_h
    w_vT = WV_SCALE * np.concatenate(
        [wT[:, h * 384 + 256 : h * 384 + 384] for h in range(NH)], axis=1
    )  # [C, F]
    w_outT = WO_SCALE * np.ascontiguousarray(np.asarray(w_out, dtype=np.float32).T)
    b_proj = np.asarray(b_proj, dtype=np.float32)
    b_qk = WQK_SCALE * np.stack(
        [
            b_proj[h * 384 + half * 128 : h * 384 + half * 128 + 128]
            for h in range(NH)
            for half in range(2)
        ],
        axis=1,
    )  # [128, 2*NH], col t matches qk tile order
    b_v = WV_SCALE * np.concatenate(
        [b_proj[h * 384 + 256 : h * 384 + 384] for h in range(NH)]
    )  # [F]
    b_v_bcast = np.broadcast_to(np.concatenate([b_v, b_v]), (128, 2 * F))
    b_out_t = np.asarray(b_out, dtype=np.float32).reshape(KT, 128).T  # [128, KT]
    esh = np.full((128, 1), ESHIFT, dtype=np.float32)
    bias = np.ascontiguousarray(
        np.concatenate([b_qk, b_v_bcast, b_out_t, esh], axis=1), dtype=np.float32
    )  # [128, 2*NH + 2*F + KT + 1]
    return x_f, np.ascontiguousarray(w_qkT), np.ascontiguousarray(w_vT), w_outT, bias


def kernel(x, w_proj, b_proj, w_out, b_out, n_heads):
    from concourse.bass_utils import run_bass_kernel_spmd

    assert int(n_heads) == NH
    x_f, w_qkT, w_vT, w_outT, bias = _prep_inputs(x, w_proj, b_proj, w_out, b_out)

    if "nc" not in _CACHE:
        _CACHE["nc"] = _build()
    nc = _CACHE["nc"]

    in_maps = [
        {
            "x": np.ascontiguousarray(x_f[c * BL : (c + 1) * BL]),
            "w_qkT": w_qkT,
            "w_vT": w_vT,
            "w_outT": w_outT,
            "bias": bias,
        }
        for c in range(NCORES)
    ]
    res = run_bass_kernel_spmd(nc, in_maps, list(range(NCORES)))
    out = np.concatenate([res.results[c]["out"] for c in range(NCORES)], axis=0)
    return out.reshape(B, C, 32, 32)


# revision 24
# speedup vs baseline: 1.0726x; 1.0726x over previous
"""AttentionBlock Trainium2 kernel (fp8-DoubleRow edition).

Reference computation (B=16, C=512, H=W=32, n_heads=4, d_k=128):
    xs   = x.reshape(B,C,S).T            # [B, S, C],  S = 1024
    qkv  = xs @ w_proj.T + b_proj        # [B, S, 1536]
    S_   = einsum('bihd,bjhd->bijh', q, k) * d_k**-0.5
    attn = softmax(S_, axis=1)           # over the QUERY axis i (source quirk)
    res  = einsum('bijh,bjhd->bihd', attn, v)
    out  = res @ w_out.T + b_out + xs    # residual
    return out.T.reshape(B, C, H, W)

Strategy: data-parallel over batch, 2 batches per core on 8 cores. Transposed
layouts avoid on-device transposes.

Precision: the output is dominated by the fp32 residual xs (+b_out); the
attention path contributes only ~8% of the output magnitude (std 0.08 vs
1.0), so it tolerates fp8. The QKV/output projections and the AV contraction
run as fp8e4 DoubleRow matmuls (K=256 per pass: measured ~207-244ns vs 270ns
for a bf16 K=128 matmul). Scores stay bf16 (contraction is only d_k=128, so
DoubleRow can't help, and it keeps exp() inputs accurate). The softmax runs
as exp on ACT with fp8 output + free accum_out row sums; the normalizer is
folded into per-(head,j-tile) fp8 V rows (v_sc = 64*v/Z) on the DVE — only
tiny ops sit in the PE->ACT->DVE->PE chain (bulk DVE ops there measurably
stall the strict-FIFO queue). fp8 range handling:
  wqk *= 16   -> qk_sb holds 16*(q|k) in bf16; exp scale absorbs the 256
  wv  *= 64   -> v_sb holds 64*v (bf16); v_sc = 64*v/Z is fp8-normal
  wo  *= 16   -> out-proj psum = 1024*(res@wo.T); epilogue applies 1/1024
  exp bias=-2 (softmax shift-invariant) keeps e^s in [~0.007, ~40] for fp8
The residual + b_out add is one fp32 scalar_tensor_tensor on DVE against a
precomputed xb = x + b_out.

Scheduling: the softmax (ACT) is the second-busiest engine after PE, so all
PE-only projection work is interleaved into the ACT-bound attention phases:
next-head QK projections after each head, the NEXT batch's QK01+V projection
during the last head, and batch 0's output projection inside batch 1's head
phases. PSUM accumulation is emitted bank-interleaved (A,B,A,B) because
back-to-back same-bank accumulating matmuls measure ~+60ns each.
"""
import sys

for _p in (
    "/opt/trn_rl_repo",
    "/root/.axon_site",
    "/root/.axon_site/_ro/trn_rl_repo",
    "/root/.axon_site/_ro/pypackages",
):
    if _p not in sys.path:
        sys.path.append(_p)

import numpy as np

B = 16
C = 512
S = 1024  # H*W
NH = 4
DK = 128
F = NH * DK  # 512
NCORES = 8
BL = B // NCORES  # batches per core
KT = C // 128  # 4  contraction tiles over channels
ST = S // 128  # 8  seq tiles
NT = S // 512  # 2  free-dim chunks of 512
SCALE = float(DK) ** -0.5
WQK_SCALE = 16.0  # host pre-scale on w_qkT/b_qk -> scores psum = 256*s
WV_SCALE = 64.0  # host pre-scale on w_vT/b_v -> racc = 64*res
WO_SCALE = 16.0  # host pre-scale on w_outT -> out psum = 1024*out_attn
ESHIFT = -2.0  # exp(s*scale + ESHIFT): softmax-invariant fp8 range shift

_CACHE: dict = {}


def _build(repeat=1, unroll=1):
    """Build the kernel. repeat>1 wraps the per-call workload in an on-device
    For_i loop — used only for timing (amortizes the ~10ms axon dispatch).
    unroll>1 emits the workload N times inline (cost-model analysis only)."""
    import contextlib

    import concourse.tile as tile
    from concourse import bacc, mybir

    F32 = mybir.dt.float32
    F32R = mybir.dt.float32r
    BF16 = mybir.dt.bfloat16
    FP8 = mybir.dt.float8e4
    EXP = mybir.ActivationFunctionType.Exp

    nc = bacc.Bacc("TRN2", debug=False)
    x_d = nc.dram_tensor("x", [BL, C, S], F32, kind="ExternalInput").ap()
    wqk_d = nc.dram_tensor("w_qkT", [C, 2 * F], F32, kind="ExternalInput").ap()
    wv_d = nc.dram_tensor("w_vT", [C, F], F32, kind="ExternalInput").ap()
    wo_d = nc.dram_tensor("w_outT", [F, C], F32, kind="ExternalInput").ap()
    bias_d = nc.dram_tensor("bias", [128, 2 * NH + 2 * F + KT + 1], F32, kind="ExternalInput").ap()
    out_d = nc.dram_tensor("out", [BL, C, S], F32, kind="ExternalOutput").ap()

    wqk_r = wqk_d.rearrange("(k p) m -> p k m", p=128)
    wv_r = wv_d.rearrange("(k p) m -> p k m", p=128)
    wo_r = wo_d.rearrange("(k p) m -> p k m", p=128)

    with tile.TileContext(nc) as tc:
        with (
            tc.tile_pool(name="const", bufs=1) as constp,
            tc.tile_pool(name="stage", bufs=1) as stagep,
            tc.tile_pool(name="xp", bufs=2) as xp,
            tc.tile_pool(name="qkp", bufs=2) as qkp,
            tc.tile_pool(name="vp", bufs=2) as vp,
            tc.tile_pool(name="ep", bufs=3) as ep,
            tc.tile_pool(name="etp", bufs=3) as etp,
            tc.tile_pool(name="rp", bufs=2) as rp,
            tc.tile_pool(name="op", bufs=2) as op,
            tc.tile_pool(name="small", bufs=16) as smallp,
            tc.tile_pool(name="vs", bufs=8) as vsp,
            # psum: pp = [128,512]x2 shared by qk/v/out projections;
            # ps = [128,1024]x2 for score tiles; pr = [128,512]x2 for the
            # per-head AV accumulators. 2+4+2 = 8 banks.
            tc.tile_pool(name="pp", bufs=2, space="PSUM") as pp,
            tc.tile_pool(name="ps", bufs=2, space="PSUM") as ps,
            tc.tile_pool(name="pr", bufs=2, space="PSUM") as pr,
        ):
            # ---- constants: load fp32, convert once to fp8 ----
            wqk8_sb = constp.tile([128, KT, 2 * F], FP8)
            wv8_sb = constp.tile([128, KT, F], FP8)
            wo8_sb = constp.tile([128, KT, C], FP8)
            bias_sb = constp.tile([128, 2 * NH + 2 * F + KT + 1], F32)
            x_sbs = [xp.tile([128, KT, S], F32, name=f"x{b}", tag="x") for b in range(BL)]
            x8_sbs = [xp.tile([128, KT, S], FP8, name=f"x8{b}", tag="x8") for b in range(BL)]
            # xb = x + b_out (residual + out bias pre-added once; the out-proj
            # epilogue is then a single scalar_tensor_tensor per half)
            xb_sbs = [xp.tile([128, KT, S], F32, name=f"xb{b}", tag="xb") for b in range(BL)]

            wqk_st = stagep.tile([128, KT, 2 * F], F32, name="wst", tag="wst")
            for k in range(KT):
                nc.sync.dma_start(out=x_sbs[0][:, k, :], in_=x_d[0, bass_ts(k, 128), :])
                nc.sync.dma_start(out=wqk_st[:, k, :], in_=wqk_r[:, k, :])
            nc.vector.tensor_copy(x8_sbs[0], x_sbs[0])
            nc.gpsimd.tensor_copy(wqk8_sb, wqk_st)
            wv_st = stagep.tile([128, KT, F], F32, name="wst", tag="wst")
            nc.sync.dma_start(out=wv_st, in_=wv_r)
            nc.gpsimd.tensor_copy(wv8_sb, wv_st)
            nc.sync.dma_start(out=bias_sb, in_=bias_d)
            for b in range(1, BL):
                for k in range(KT):
                    nc.sync.dma_start(out=x_sbs[b][:, k, :], in_=x_d[b, bass_ts(k, 128), :])
                nc.vector.tensor_copy(x8_sbs[b], x_sbs[b])
            wo_st = stagep.tile([128, KT, C], F32, name="wst", tag="wst")
            nc.sync.dma_start(out=wo_st, in_=wo_r)
            nc.gpsimd.tensor_copy(wo8_sb, wo_st)
            for b in range(BL):
                for k in range(KT):
                    nc.gpsimd.tensor_scalar_add(
                        xb_sbs[b][:, k, :],
                        x_sbs[b][:, k, :],
                        bias_sb[:, 2 * NH + 2 * F + k : 2 * NH + 2 * F + k + 1],
                    )
            b_qk = bias_sb[:, 0 : 2 * NH]  # per-partition bias per qk f-tile (x16)
            b_v2 = bias_sb[:, 2 * NH : 2 * NH + 2 * F]  # v bias doubled (x64)
            b_esh = bias_sb[:, 2 * NH + 2 * F + KT :]  # ESHIFT constant column

            env = dict(
                nc=nc, qkp=qkp, vp=vp, ep=ep, etp=etp, rp=rp, op=op, smallp=smallp,
                vsp=vsp, pp=pp, ps=ps, pr=pr, wqk8_sb=wqk8_sb, wv8_sb=wv8_sb,
                wo8_sb=wo8_sb, b_qk=b_qk, b_v2=b_v2, b_esh=b_esh, out_d=out_d,
                x8_sbs=x8_sbs, xb_sbs=xb_sbs, F32=F32, F32R=F32R, BF16=BF16, FP8=FP8,
                EXP=EXP, mybir=mybir,
            )
            # software-pipeline prologue: batch 0's QK01 + V projections
            pend = {0: _proj_phase(env, 0)}

            rep_ctx = (
                tc.For_i(0, repeat, 1) if repeat > 1 else contextlib.nullcontext()
            )
            with rep_ctx:
                for _u in range(unroll):
                    _batches(env, pend)

    nc.compile()
    return nc


def _proj_phase(env, b):
    """QK f-tiles 0,1 + full V projection for batch b. Returns (qk_sb, v_sb)."""
    nc = env["nc"]
    F32, BF16 = env["F32"], env["BF16"]
    x8 = env["x8_sbs"][b]
    pp, b_qk, b_v2 = env["pp"], env["b_qk"], env["b_v2"]
    wv8_sb = env["wv8_sb"]

    qk_sb, v_sb, chunks = _proj_phase_chunks(env, b)
    for c in chunks:
        c()
    return qk_sb, v_sb


def _proj_phase_chunks(env, b):
    """Like _proj_phase but returns the work as closures (PE filler chunks)."""
    nc = env["nc"]
    F32 = env["F32"]
    x8 = env["x8_sbs"][b]
    pp, b_v2 = env["pp"], env["b_v2"]
    wv8_sb = env["wv8_sb"]

    qk_sb = env["qkp"].tile([128, 2 * NH, S], env["BF16"], name=f"qk{b}", tag="qk")
    v_sb = env["vp"].tile([128, ST, F], env["BF16"], name=f"v{b}", tag="v")

    def vchunk(stp):
        st, st2 = 2 * stp, 2 * stp + 1
        acc_a = pp.tile([128, 512], F32, name="va", tag="pp")
        acc_b = pp.tile([128, 512], F32, name="vb", tag="pp")
        for k2 in range(2):
            nc.tensor.matmul(
                acc_a,
                x8[:, 2 * k2 : 2 * k2 + 2, bass_ts(st, 128)],
                wv8_sb[:, 2 * k2 : 2 * k2 + 2, :],
                start=(k2 == 0),
                stop=(k2 == 1),
                perf_mode=_dr(),
            )
            nc.tensor.matmul(
                acc_b,
                x8[:, 2 * k2 : 2 * k2 + 2, bass_ts(st2, 128)],
                wv8_sb[:, 2 * k2 : 2 * k2 + 2, :],
                start=(k2 == 0),
                stop=(k2 == 1),
                perf_mode=_dr(),
            )
        nc.vector.tensor_add(v_sb[:, st, :], acc_a, b_v2[:, 0:F])
        nc.vector.tensor_add(v_sb[:, st2, :], acc_b, b_v2[:, 0:F])

    chunks = [
        (lambda n=n: _qk_proj_chunk(env, x8, qk_sb, 0, 1, n)) for n in range(NT)
    ] + [(lambda stp=stp: vchunk(stp)) for stp in range(ST // 2)]
    return qk_sb, v_sb, chunks


def _qk_proj(env, x8, qk_sb, t, t2):
    for n in range(NT):
        _qk_proj_chunk(env, x8, qk_sb, t, t2, n)


def _qk_proj_chunk(env, x8, qk_sb, t, t2, n):
    # Q^T/K^T f-tiles t,t2, one 512-column chunk: qk = 16*w_qkT.T @ x
    # Two DoubleRow k-blocks (channels 0:256, 256:512), A/B bank
    # interleaved so same-bank accumulating MMs are 2 apart.
    nc = env["nc"]
    F32 = env["F32"]
    pp, wqk8_sb, b_qk = env["pp"], env["wqk8_sb"], env["b_qk"]
    if True:
        acc_a = pp.tile([128, 512], F32, name="qka", tag="pp")
        acc_b = pp.tile([128, 512], F32, name="qkb", tag="pp")
        for k2 in range(2):
            nc.tensor.matmul(
                acc_a,
                wqk8_sb[:, 2 * k2 : 2 * k2 + 2, bass_ts(t, 128)],
                x8[:, 2 * k2 : 2 * k2 + 2, bass_ts(n, 512)],
                start=(k2 == 0),
                stop=(k2 == 1),
                perf_mode=_dr(),
            )
            nc.tensor.matmul(
                acc_b,
                wqk8_sb[:, 2 * k2 : 2 * k2 + 2, bass_ts(t2, 128)],
                x8[:, 2 * k2 : 2 * k2 + 2, bass_ts(n, 512)],
                start=(k2 == 0),
                stop=(k2 == 1),
                perf_mode=_dr(),
            )
        nc.vector.tensor_scalar_add(
            qk_sb[:, t, bass_ts(n, 512)], acc_a, b_qk[:, t : t + 1]
        )
        nc.vector.tensor_scalar_add(
            qk_sb[:, t2, bass_ts(n, 512)], acc_b, b_qk[:, t2 : t2 + 1]
        )


def _out_proj_chunk(env, b, ct, resT_sb):
    """Output projection c-tile ct for batch b + fused epilogue + store."""
    nc = env["nc"]
    F32, mybir = env["F32"], env["mybir"]
    pp, wo8_sb, out_d = env["pp"], env["wo8_sb"], env["out_d"]
    xb = env["xb_sbs"][b]

    out_t = env["op"].tile([128, S], F32, name="ot", tag="ot")
    accs = [pp.tile([128, 512], F32, name=f"oa{n}", tag="pp") for n in range(NT)]
    for f2 in range(2):
        for n in range(NT):
            nc.tensor.matmul(
                accs[n],
                wo8_sb[:, 2 * f2 : 2 * f2 + 2, bass_ts(ct, 128)],
                resT_sb[:, 2 * f2 : 2 * f2 + 2, bass_ts(n, 512)],
                start=(f2 == 0),
                stop=(f2 == 1),
                perf_mode=_dr(),
            )
    # fused epilogue: out = acc/1024 + (x + b_out), then store per half.
    # (must be DVE: GPSIMD cannot read PSUM)
    for n in range(NT):
        nc.vector.scalar_tensor_tensor(
            out_t[:, bass_ts(n, 512)],
            accs[n],
            1.0 / (WV_SCALE * WO_SCALE),
            xb[:, ct, bass_ts(n, 512)],
            mybir.AluOpType.mult,
            mybir.AluOpType.add,
        )
        nc.sync.dma_start(
            out=out_d[b, bass_ts(ct, 128), bass_ts(n, 512)],
            in_=out_t[:, bass_ts(n, 512)],
        )


def _batches(env, pend):
    """One full workload pass (both batches), software-pipelined.

    Batch b's attention phases interleave: next-head QK tiles (h0-h2), the
    NEXT batch's QK01+V projection (h3), and for b=1 the PREVIOUS batch's
    output projection (one c-tile per head phase). Batch 1's own output
    projection runs at the end (the only PE-only stretch left).
    """
    nc = env["nc"]
    F32, FP8, EXP = env["F32"], env["FP8"], env["EXP"]
    ep, rp, smallp, vsp = env["ep"], env["rp"], env["smallp"], env["vsp"]
    ps, pr = env["ps"], env["pr"]
    b_esh = env["b_esh"]

    resT = {}
    for b in range(BL):
        qk_sb, v_sb = pend.pop(b)
        x8_next = env["x8_sbs"][(b + 1) % BL]

        resT_sb = rp.tile([128, NH, S], FP8, name=f"resT{b}", tag="resT")
        resT[b] = resT_sb
        for h in range(NH):
            # PE-only filler chunks for this ACT-bound phase. They are
            # emitted BETWEEN the score groups (engines are strict FIFO, so
            # filler parked at the phase end would leave PE stalled at each
            # AV waiting on exp, and would delay the next phase's scores).
            fillers = []
            if h + 1 < NH:
                fillers += [
                    (lambda n=n: _qk_proj_chunk(
                        env, env["x8_sbs"][b], qk_sb, 2 * h + 2, 2 * h + 3, n))
                    for n in range(NT)
                ]
            else:
                # last head: project the next batch (next iteration's b=0
                # when b is the last batch — recomputed there, harmless)
                qk_n, v_n, chunks = _proj_phase_chunks(env, (b + 1) % BL)
                pend[(b + 1) % BL] = (qk_n, v_n)
                fillers += chunks
            if b == BL - 1:
                # batch 0's output projection, one c-tile per head phase
                fillers.append(lambda: _out_proj_chunk(env, 0, h, resT[0]))

            racc = [pr.tile([128, 512], F32, name=f"racc{n}", tag="racc") for n in range(NT)]
            av_pend = None  # AV is delayed one j-tile-pair for exp grace
            popped = 0
            for jtp in range(ST // 2):
                e2 = ep.tile([128, 2, S], FP8, name="e2", tag="e2")
                v_sc2 = vsp.tile([128, 2, DK], FP8, name="vsc", tag="vsc")
                for half in range(2):
                    jt = 2 * jtp + half
                    ssum = smallp.tile([128, 2], F32, name="ssum", tag="ssum")
                    # scores S^T[j, i] for one j-tile: [128, 1024] PSUM
                    # (2 banks); one exp pass over both halves, softmax
                    # denominator via accum_out. psum holds 256*s.
                    sacc = ps.tile([128, S], F32, name="sacc", tag="sacc")
                    for n in range(NT):
                        nc.tensor.matmul(
                            sacc[:, bass_ts(n, 512)],
                            qk_sb[:, 2 * h + 1, bass_ts(jt, 128)],
                            qk_sb[:, 2 * h, bass_ts(n, 512)],
                            start=True,
                            stop=True,
                        )
                    nc.scalar.activation(
                        out=e2[:, half, :],
                        in_=sacc,
                        func=EXP,
                        scale=SCALE / (WQK_SCALE * WQK_SCALE),
                        bias=b_esh,
                        accum_out=ssum[:, 0:1],
                    )
                    nc.vector.reciprocal(ssum[:, 1:2], ssum[:, 0:1])
                    nc.vector.tensor_scalar_mul(
                        v_sc2[:, half, :],
                        v_sb[:, jt, bass_ts(h, DK)],
                        ssum[:, 1:2],
                    )
                # round-robin filler between score groups
                want = len(fillers) * (jtp + 1) // (ST // 2)
                while popped < want:
                    fillers[popped]()
                    popped += 1
                if av_pend is not None:
                    _av_mms(env, racc, av_pend[0], av_pend[1], av_pend[2])
                av_pend = (v_sc2, e2, jtp)
            while popped < len(fillers):
                fillers[popped]()
                popped += 1
            _av_mms(env, racc, av_pend[0], av_pend[1], av_pend[2])
            for n in range(NT):
                nc.vector.tensor_copy(
                    resT_sb[:, h, bass_ts(n, 512)], racc[n]
                )

    # ---- batch 1's output projection (tail) ----
    for ct in range(KT):
        _out_proj_chunk(env, BL - 1, ct, resT[BL - 1])


def _av_mms(env, racc, v_sc2, e2, jtp):
    nc = env["nc"]
    for n in range(NT):
        nc.tensor.matmul(
            racc[n],
            v_sc2,
            e2[:, :, bass_ts(n, 512)],
            start=(jtp == 0),
            stop=(jtp == ST // 2 - 1),
            perf_mode=_dr(),
        )


def _dr():
    from concourse import mybir

    return mybir.MatmulPerfMode.DoubleRow


def bass_ts(i, size):
    import concourse.bass as bass

    return bass.ts(i, size)


def _prep_inputs(x, w_proj, b_proj, w_out, b_out):
    """Host-side reshaping into the layouts the kernel expects."""
    x_f = np.ascontiguousarray(x.reshape(B, C, S), dtype=np.float32)
    wT = np.asarray(w_proj, dtype=np.float32).T  # [C, 3*F], f = h*384 + j
    w_qkT = WQK_SCALE * np.concatenate(
        [wT[:, h * 384 : h * 384 + 256] for h in range(NH)], axis=1
    )  # [C, 2F]; col tile t=2h -> q_h, t=2h+1 -> k_h
    w_vT = WV_SCALE * np.concatenate(
        [wT[:, h * 384 + 256 : h * 384 + 384] for h in range(NH)], axis=1
    )  # [C, F]
    w_outT = WO_SCALE * np.ascontiguousarray(np.asarray(w_out, dtype=np.float32).T)
    b_proj = np.asarray(b_proj, dtype=np.float32)
    b_qk = WQK_SCALE * np.stack(
        [
            b_proj[h * 384 + half * 128 : h * 384 + half * 128 + 128]
            for h in range(NH)
            for half in range(2)
        ],
        axis=1,
    )  # [128, 2*NH], col t matches qk tile order
    b_v = WV_SCALE * np.concatenate(
        [b_proj[h * 384 + 256 : h * 384 + 384] for h in range(NH)]
    )  # [F]
    b_v_bcast = np.broadcast_to(np.concatenate([b_v, b_v]), (128, 2 * F))
    b_out_t = np.asarray(b_out, dtype=np.float32).reshape(KT, 128).T  # [128, KT]
    esh = np.full((128, 1), ESHIFT, dtype=np.float32)
    bias = np.ascontiguousarray(
        np.concatenate([b_qk, b_v_bcast, b_out_t, esh], axis=1), dtype=np.float32
    )  # [128, 2*NH + 2*F + KT + 1]
    return x_f, np.ascontiguousarray(w_qkT), np.ascontiguousarray(w_vT), w_outT, bias


def kernel(x, w_proj, b_proj, w_out, b_out, n_heads):
    from concourse.bass_utils import run_bass_kernel_spmd

    assert int(n_heads) == NH
    x_f, w_qkT, w_vT, w_outT, bias = _prep_inputs(x, w_proj, b_proj, w_out, b_out)

    if "nc" not in _CACHE:
        _CACHE["nc"] = _build()
    nc = _CACHE["nc"]

    in_maps = [
        {
            "x": np.ascontiguousarray(x_f[c * BL : (c + 1) * BL]),
            "w_qkT": w_qkT,
            "w_vT": w_vT,
            "w_outT": w_outT,
            "bias": bias,
        }
        for c in range(NCORES)
    ]
    res = run_bass_kernel_spmd(nc, in_maps, list(range(NCORES)))
    out = np.concatenate([res.results[c]["out"] for c in range(NCORES)], axis=0)
    return out.reshape(B, C, 32, 32)


# revision 25
# speedup vs baseline: 1.6529x; 1.5410x over previous
"""AttentionBlock Trainium2 kernel (fp8-DoubleRow edition).

Reference computation (B=16, C=512, H=W=32, n_heads=4, d_k=128):
    xs   = x.reshape(B,C,S).T            # [B, S, C],  S = 1024
    qkv  = xs @ w_proj.T + b_proj        # [B, S, 1536]
    S_   = einsum('bihd,bjhd->bijh', q, k) * d_k**-0.5
    attn = softmax(S_, axis=1)           # over the QUERY axis i (source quirk)
    res  = einsum('bijh,bjhd->bihd', attn, v)
    out  = res @ w_out.T + b_out + xs    # residual
    return out.T.reshape(B, C, H, W)

Strategy: data-parallel over batch, 2 batches per core on 8 cores. Transposed
layouts avoid on-device transposes.

Precision: the output is dominated by the fp32 residual xs (+b_out); the
attention path contributes only ~8% of the output magnitude (std 0.08 vs
1.0), so it tolerates fp8. The QKV/output projections and the AV contraction
run as fp8e4 DoubleRow matmuls (K=256 per pass: measured ~207-244ns vs 270ns
for a bf16 K=128 matmul). Scores stay bf16 (contraction is only d_k=128, so
DoubleRow can't help, and it keeps exp() inputs accurate). The softmax runs
as exp on ACT with fp8 output + free accum_out row sums; the normalizer is
folded into per-(head,j-tile) fp8 V rows (v_sc = 64*v/Z) on the DVE — only
tiny ops sit in the PE->ACT->DVE->PE chain (bulk DVE ops there measurably
stall the strict-FIFO queue). fp8 range handling:
  wqk *= 16   -> qk_sb holds 16*(q|k) in bf16; exp scale absorbs the 256
  wv  *= 64   -> v_sb holds 64*v (bf16); v_sc = 64*v/Z is fp8-normal
  wo  *= 16   -> out-proj psum = 1024*(res@wo.T); epilogue applies 1/1024
  exp bias=-2 (softmax shift-invariant) keeps e^s in [~0.007, ~40] for fp8
The residual + b_out add is one fp32 scalar_tensor_tensor on DVE against a
precomputed xb = x + b_out.

Scheduling: the softmax (ACT) is the second-busiest engine after PE, so all
PE-only projection work is interleaved into the ACT-bound attention phases:
next-head QK projections after each head, the NEXT batch's QK01+V projection
during the last head, and batch 0's output projection inside batch 1's head
phases. PSUM accumulation is emitted bank-interleaved (A,B,A,B) because
back-to-back same-bank accumulating matmuls measure ~+60ns each.
"""
import sys

for _p in (
    "/opt/trn_rl_repo",
    "/root/.axon_site",
    "/root/.axon_site/_ro/trn_rl_repo",
    "/root/.axon_site/_ro/pypackages",
):
    if _p not in sys.path:
        sys.path.append(_p)

import numpy as np

B = 16
C = 512
S = 1024  # H*W
NH = 4
DK = 128
F = NH * DK  # 512
NCORES = 8
BL = B // NCORES  # batches per core
KT = C // 128  # 4  contraction tiles over channels
ST = S // 128  # 8  seq tiles
NT = S // 512  # 2  free-dim chunks of 512
SCALE = float(DK) ** -0.5
WQK_SCALE = 16.0  # host pre-scale on w_qkT/b_qk -> scores psum = 256*s
WV_SCALE = 64.0  # host pre-scale on w_vT/b_v -> racc = 64*res
WO_SCALE = 16.0  # host pre-scale on w_outT -> out psum = 1024*out_attn
ESHIFT = -2.0  # exp(s*scale + ESHIFT): softmax-invariant fp8 range shift

_CACHE: dict = {}


def _build(repeat=1, unroll=1):
    """Build the kernel. repeat>1 wraps the per-call workload in an on-device
    For_i loop — used only for timing (amortizes the ~10ms axon dispatch).
    unroll>1 emits the workload N times inline (cost-model analysis only)."""
    import contextlib

    import concourse.tile as tile
    from concourse import bacc, mybir

    F32 = mybir.dt.float32
    F32R = mybir.dt.float32r
    BF16 = mybir.dt.bfloat16
    FP8 = mybir.dt.float8e4
    EXP = mybir.ActivationFunctionType.Exp

    nc = bacc.Bacc("TRN2", debug=False)
    x_d = nc.dram_tensor("x", [BL, C, S], F32, kind="ExternalInput").ap()
    wqk_d = nc.dram_tensor("w_qkT", [C, 2 * F], F32, kind="ExternalInput").ap()
    wv_d = nc.dram_tensor("w_vT", [C, F], F32, kind="ExternalInput").ap()
    wo_d = nc.dram_tensor("w_outT", [F, C], F32, kind="ExternalInput").ap()
    bias_d = nc.dram_tensor("bias", [128, 2 * NH + 2 * F + KT + 1], F32, kind="ExternalInput").ap()
    out_d = nc.dram_tensor("out", [BL, C, S], F32, kind="ExternalOutput").ap()

    wqk_r = wqk_d.rearrange("(k p) m -> p k m", p=128)
    wv_r = wv_d.rearrange("(k p) m -> p k m", p=128)
    wo_r = wo_d.rearrange("(k p) m -> p k m", p=128)

    with tile.TileContext(nc) as tc:
        with (
            tc.tile_pool(name="const", bufs=1) as constp,
            tc.tile_pool(name="stage", bufs=1) as stagep,
            tc.tile_pool(name="xp", bufs=2) as xp,
            tc.tile_pool(name="qkp", bufs=2) as qkp,
            tc.tile_pool(name="vp", bufs=2) as vp,
            tc.tile_pool(name="ep", bufs=3) as ep,
            tc.tile_pool(name="etp", bufs=3) as etp,
            tc.tile_pool(name="rp", bufs=2) as rp,
            tc.tile_pool(name="op", bufs=2) as op,
            tc.tile_pool(name="small", bufs=16) as smallp,
            tc.tile_pool(name="vs", bufs=8) as vsp,
            # psum: pp = [128,512]x2 shared by qk/v/out projections;
            # ps = [128,1024]x2 for score tiles; pr = [128,512]x2 for the
            # per-head AV accumulators. 2+4+2 = 8 banks.
            tc.tile_pool(name="pp", bufs=2, space="PSUM") as pp,
            tc.tile_pool(name="ps", bufs=2, space="PSUM") as ps,
            tc.tile_pool(name="pr", bufs=2, space="PSUM") as pr,
        ):
            # ---- constants: load fp32, convert once to fp8 ----
            wqk8_sb = constp.tile([128, KT, 2 * F], FP8)
            wv8_sb = constp.tile([128, KT, F], FP8)
            wo8_sb = constp.tile([128, KT, C], FP8)
            bias_sb = constp.tile([128, 2 * NH + 2 * F + KT + 1], F32)
            x_sbs = [xp.tile([128, KT, S], F32, name=f"x{b}", tag="x") for b in range(BL)]
            x8_sbs = [xp.tile([128, KT, S], FP8, name=f"x8{b}", tag="x8") for b in range(BL)]
            # xb = x + b_out (residual + out bias pre-added once; the out-proj
            # epilogue is then a single scalar_tensor_tensor per half)
            xb_sbs = [xp.tile([128, KT, S], F32, name=f"xb{b}", tag="xb") for b in range(BL)]

            wqk_st = stagep.tile([128, KT, 2 * F], F32, name="wst", tag="wst")
            for k in range(KT):
                nc.sync.dma_start(out=x_sbs[0][:, k, :], in_=x_d[0, bass_ts(k, 128), :])
                nc.sync.dma_start(out=wqk_st[:, k, :], in_=wqk_r[:, k, :])
            nc.vector.tensor_copy(x8_sbs[0], x_sbs[0])
            nc.gpsimd.tensor_copy(wqk8_sb, wqk_st)
            wv_st = stagep.tile([128, KT, F], F32, name="wst", tag="wst")
            nc.sync.dma_start(out=wv_st, in_=wv_r)
            nc.gpsimd.tensor_copy(wv8_sb, wv_st)
            nc.sync.dma_start(out=bias_sb, in_=bias_d)
            for b in range(1, BL):
                for k in range(KT):
                    nc.sync.dma_start(out=x_sbs[b][:, k, :], in_=x_d[b, bass_ts(k, 128), :])
                nc.vector.tensor_copy(x8_sbs[b], x_sbs[b])
            wo_st = stagep.tile([128, KT, C], F32, name="wst", tag="wst")
            nc.sync.dma_start(out=wo_st, in_=wo_r)
            nc.gpsimd.tensor_copy(wo8_sb, wo_st)
            for b in range(BL):
                for k in range(KT):
                    nc.gpsimd.tensor_scalar_add(
                        xb_sbs[b][:, k, :],
                        x_sbs[b][:, k, :],
                        bias_sb[:, 2 * NH + 2 * F + k : 2 * NH + 2 * F + k + 1],
                    )
            b_qk = bias_sb[:, 0 : 2 * NH]  # per-partition bias per qk f-tile (x16)
            b_v2 = bias_sb[:, 2 * NH : 2 * NH + 2 * F]  # v bias doubled (x64)
            b_esh = bias_sb[:, 2 * NH + 2 * F + KT :]  # ESHIFT constant column

            env = dict(
                nc=nc, qkp=qkp, vp=vp, ep=ep, etp=etp, rp=rp, op=op, smallp=smallp,
                vsp=vsp, pp=pp, ps=ps, pr=pr, wqk8_sb=wqk8_sb, wv8_sb=wv8_sb,
                wo8_sb=wo8_sb, b_qk=b_qk, b_v2=b_v2, b_esh=b_esh, out_d=out_d,
                x8_sbs=x8_sbs, xb_sbs=xb_sbs, F32=F32, F32R=F32R, BF16=BF16, FP8=FP8,
                EXP=EXP, mybir=mybir,
            )
            # software-pipeline prologue: batch 0's QK01 + V projections
            pend = {0: _proj_phase(env, 0)}

            rep_ctx = (
                tc.For_i(0, repeat, 1) if repeat > 1 else contextlib.nullcontext()
            )
            with rep_ctx:
                for _u in range(unroll):
                    _batches(env, pend)

    nc.compile()
    return nc


def _proj_phase(env, b):
    """QK f-tiles 0,1 + full V projection for batch b. Returns (qk_sb, v_sb)."""
    nc = env["nc"]
    F32, BF16 = env["F32"], env["BF16"]
    x8 = env["x8_sbs"][b]
    pp, b_qk, b_v2 = env["pp"], env["b_qk"], env["b_v2"]
    wv8_sb = env["wv8_sb"]

    qk_sb, v_sb, chunks = _proj_phase_chunks(env, b)
    for c in chunks:
        c()
    return qk_sb, v_sb


def _proj_phase_chunks(env, b):
    """Like _proj_phase but returns the work as closures (PE filler chunks)."""
    nc = env["nc"]
    F32 = env["F32"]
    x8 = env["x8_sbs"][b]
    pp, b_v2 = env["pp"], env["b_v2"]
    wv8_sb = env["wv8_sb"]

    qk_sb = env["qkp"].tile([128, 2 * NH, S], env["BF16"], name=f"qk{b}", tag="qk")
    v_sb = env["vp"].tile([128, ST, F], env["BF16"], name=f"v{b}", tag="v")

    def vchunk(stp):
        st, st2 = 2 * stp, 2 * stp + 1
        acc_a = pp.tile([128, 512], F32, name="va", tag="pp")
        acc_b = pp.tile([128, 512], F32, name="vb", tag="pp")
        for k2 in range(2):
            nc.tensor.matmul(
                acc_a,
                x8[:, 2 * k2 : 2 * k2 + 2, bass_ts(st, 128)],
                wv8_sb[:, 2 * k2 : 2 * k2 + 2, :],
                start=(k2 == 0),
                stop=(k2 == 1),
                perf_mode=_dr(),
            )
            nc.tensor.matmul(
                acc_b,
                x8[:, 2 * k2 : 2 * k2 + 2, bass_ts(st2, 128)],
                wv8_sb[:, 2 * k2 : 2 * k2 + 2, :],
                start=(k2 == 0),
                stop=(k2 == 1),
                perf_mode=_dr(),
            )
        nc.vector.tensor_add(v_sb[:, st, :], acc_a, b_v2[:, 0:F])
        nc.vector.tensor_add(v_sb[:, st2, :], acc_b, b_v2[:, 0:F])

    chunks = [
        (lambda n=n: _qk_proj_chunk(env, x8, qk_sb, 0, 1, n)) for n in range(NT)
    ] + [(lambda stp=stp: vchunk(stp)) for stp in range(ST // 2)]
    return qk_sb, v_sb, chunks


def _qk_proj(env, x8, qk_sb, t, t2):
    for n in range(NT):
        _qk_proj_chunk(env, x8, qk_sb, t, t2, n)


def _qk_proj_chunk(env, x8, qk_sb, t, t2, n):
    # Q^T/K^T f-tiles t,t2, one 512-column chunk: qk = 16*w_qkT.T @ x
    # Two DoubleRow k-blocks (channels 0:256, 256:512), A/B bank
    # interleaved so same-bank accumulating MMs are 2 apart.
    nc = env["nc"]
    F32 = env["F32"]
    pp, wqk8_sb, b_qk = env["pp"], env["wqk8_sb"], env["b_qk"]
    if True:
        acc_a = pp.tile([128, 512], F32, name="qka", tag="pp")
        acc_b = pp.tile([128, 512], F32, name="qkb", tag="pp")
        for k2 in range(2):
            nc.tensor.matmul(
                acc_a,
                wqk8_sb[:, 2 * k2 : 2 * k2 + 2, bass_ts(t, 128)],
                x8[:, 2 * k2 : 2 * k2 + 2, bass_ts(n, 512)],
                start=(k2 == 0),
                stop=(k2 == 1),
                perf_mode=_dr(),
            )
            nc.tensor.matmul(
                acc_b,
                wqk8_sb[:, 2 * k2 : 2 * k2 + 2, bass_ts(t2, 128)],
                x8[:, 2 * k2 : 2 * k2 + 2, bass_ts(n, 512)],
                start=(k2 == 0),
                stop=(k2 == 1),
                perf_mode=_dr(),
            )
        nc.vector.tensor_scalar_add(
            qk_sb[:, t, bass_ts(n, 512)], acc_a, b_qk[:, t : t + 1]
        )
        nc.vector.tensor_scalar_add(
            qk_sb[:, t2, bass_ts(n, 512)], acc_b, b_qk[:, t2 : t2 + 1]
        )


def _out_proj_chunk(env, b, ct, resT_sb):
    """Output projection c-tile ct for batch b + fused epilogue + store."""
    nc = env["nc"]
    F32, mybir = env["F32"], env["mybir"]
    pp, wo8_sb, out_d = env["pp"], env["wo8_sb"], env["out_d"]
    xb = env["xb_sbs"][b]

    out_t = env["op"].tile([128, S], F32, name="ot", tag="ot")
    accs = [pp.tile([128, 512], F32, name=f"oa{n}", tag="pp") for n in range(NT)]
    for f2 in range(2):
        for n in range(NT):
            nc.tensor.matmul(
                accs[n],
                wo8_sb[:, 2 * f2 : 2 * f2 + 2, bass_ts(ct, 128)],
                resT_sb[:, 2 * f2 : 2 * f2 + 2, bass_ts(n, 512)],
                start=(f2 == 0),
                stop=(f2 == 1),
                perf_mode=_dr(),
            )
    # fused epilogue: out = acc/1024 + (x + b_out), then store per half.
    # (must be DVE: GPSIMD cannot read PSUM)
    for n in range(NT):
        nc.vector.scalar_tensor_tensor(
            out_t[:, bass_ts(n, 512)],
            accs[n],
            1.0 / (WV_SCALE * WO_SCALE),
            xb[:, ct, bass_ts(n, 512)],
            mybir.AluOpType.mult,
            mybir.AluOpType.add,
        )
        nc.sync.dma_start(
            out=out_d[b, bass_ts(ct, 128), bass_ts(n, 512)],
            in_=out_t[:, bass_ts(n, 512)],
        )


def _batches(env, pend):
    """One full workload pass (both batches), software-pipelined.

    Batch b's attention phases interleave: next-head QK tiles (h0-h2), the
    NEXT batch's QK01+V projection (h3), and for b=1 the PREVIOUS batch's
    output projection (one c-tile per head phase). Batch 1's own output
    projection runs at the end (the only PE-only stretch left).
    """
    nc = env["nc"]
    F32, FP8, EXP = env["F32"], env["FP8"], env["EXP"]
    ep, rp, smallp, vsp = env["ep"], env["rp"], env["smallp"], env["vsp"]
    ps, pr = env["ps"], env["pr"]
    b_esh = env["b_esh"]

    resT = {}
    for b in range(BL):
        qk_sb, v_sb = pend.pop(b)
        x8_next = env["x8_sbs"][(b + 1) % BL]

        resT_sb = rp.tile([128, NH, S], FP8, name=f"resT{b}", tag="resT")
        resT[b] = resT_sb
        for h in range(NH):
            # PE-only filler chunks for this ACT-bound phase. They are
            # emitted BETWEEN the score groups (engines are strict FIFO, so
            # filler parked at the phase end would leave PE stalled at each
            # AV waiting on exp, and would delay the next phase's scores).
            fillers = []
            if h + 1 < NH:
                fillers += [
                    (lambda n=n: _qk_proj_chunk(
                        env, env["x8_sbs"][b], qk_sb, 2 * h + 2, 2 * h + 3, n))
                    for n in range(NT)
                ]
            else:
                # last head: project the next batch (next iteration's b=0
                # when b is the last batch — recomputed there, harmless)
                qk_n, v_n, chunks = _proj_phase_chunks(env, (b + 1) % BL)
                pend[(b + 1) % BL] = (qk_n, v_n)
                fillers += chunks
            if b == BL - 1:
                # batch 0's output projection, one c-tile per head phase
                fillers.append(lambda: _out_proj_chunk(env, 0, h, resT[0]))

            racc = [pr.tile([128, 512], F32, name=f"racc{n}", tag="racc") for n in range(NT)]
            av_pend = None  # AV is delayed one j-tile-pair for exp grace
            popped = 0
            for jtp in range(ST // 2):
                e2 = ep.tile([128, 2, S], FP8, name="e2", tag="e2")
                v_sc2 = vsp.tile([128, 2, DK], FP8, name="vsc", tag="vsc")
                for half in range(2):
                    jt = 2 * jtp + half
                    ssum = smallp.tile([128, 2], F32, name="ssum", tag="ssum")
                    # scores S^T[j, i] for one j-tile: [128, 1024] PSUM
                    # (2 banks); one exp pass over both halves, softmax
                    # denominator via accum_out. psum holds 256*s.
                    sacc = ps.tile([128, S], F32, name="sacc", tag="sacc")
                    for n in range(NT):
                        nc.tensor.matmul(
                            sacc[:, bass_ts(n, 512)],
                            qk_sb[:, 2 * h + 1, bass_ts(jt, 128)],
                            qk_sb[:, 2 * h, bass_ts(n, 512)],
                            start=True,
                            stop=True,
                        )
                    if half == 0:
                        # first exp of the pair: f32r output (ACT's cheap
                        # write mode, ~1.1us vs ~1.55us direct-fp8) + plain
                        # DVE copy to fp8. Its AV consumer is a full score
                        # group away (delayed-AV), so the extra DVE hop has
                        # pipeline grace. Rebalances ACT ~99->85us while DVE
                        # stays under PE's ~81us.
                        e_t = env["etp"].tile([128, S], env["F32R"], name="et", tag="et")
                        nc.scalar.activation(
                            out=e_t,
                            in_=sacc,
                            func=EXP,
                            scale=SCALE / (WQK_SCALE * WQK_SCALE),
                            bias=b_esh,
                            accum_out=ssum[:, 0:1],
                        )
                        nc.vector.tensor_copy(e2[:, 0, :], e_t)
                    else:
                        nc.scalar.activation(
                            out=e2[:, half, :],
                            in_=sacc,
                            func=EXP,
                            scale=SCALE / (WQK_SCALE * WQK_SCALE),
                            bias=b_esh,
                            accum_out=ssum[:, 0:1],
                        )
                    nc.vector.reciprocal(ssum[:, 1:2], ssum[:, 0:1])
                    nc.vector.tensor_scalar_mul(
                        v_sc2[:, half, :],
                        v_sb[:, jt, bass_ts(h, DK)],
                        ssum[:, 1:2],
                    )
                # round-robin filler between score groups
                want = len(fillers) * (jtp + 1) // (ST // 2)
                while popped < want:
                    fillers[popped]()
                    popped += 1
                if av_pend is not None:
                    _av_mms(env, racc, av_pend[0], av_pend[1], av_pend[2])
                av_pend = (v_sc2, e2, jtp)
            while popped < len(fillers):
                fillers[popped]()
                popped += 1
            _av_mms(env, racc, av_pend[0], av_pend[1], av_pend[2])
            for n in range(NT):
                nc.vector.tensor_copy(
                    resT_sb[:, h, bass_ts(n, 512)], racc[n]
                )

    # ---- batch 1's output projection (tail) ----
    for ct in range(KT):
        _out_proj_chunk(env, BL - 1, ct, resT[BL - 1])


def _av_mms(env, racc, v_sc2, e2, jtp):
    nc = env["nc"]
    for n in range(NT):
        nc.tensor.matmul(
            racc[n],
            v_sc2,
            e2[:, :, bass_ts(n, 512)],
            start=(jtp == 0),
            stop=(jtp == ST // 2 - 1),
            perf_mode=_dr(),
        )


def _dr():
    from concourse import mybir

    return mybir.MatmulPerfMode.DoubleRow


def bass_ts(i, size):
    import concourse.bass as bass

    return bass.ts(i, size)


def _prep_inputs(x, w_proj, b_proj, w_out, b_out):
    """Host-side reshaping into the layouts the kernel expects."""
    x_f = np.ascontiguousarray(x.reshape(B, C, S), dtype=np.float32)
    wT = np.asarray(w_proj, dtype=np.float32).T  # [C, 3*F], f = h*384 + j
    w_qkT = WQK_SCALE * np.concatenate(
        [wT[:, h * 384 : h * 384 + 256] for h in range(NH)], axis=1
    )  # [C, 2F]; col tile t=2h -> q_h, t=2h+1 -> k_h
    w_vT = WV_SCALE * np.concatenate(
        [wT[:, h * 384 + 256 : h * 384 + 384] for h in range(NH)], axis=1
    )  # [C, F]
    w_outT = WO_SCALE * np.ascontiguousarray(np.asarray(w_out, dtype=np.float32).T)
    b_proj = np.asarray(b_proj, dtype=np.float32)
    b_qk = WQK_SCALE * np.stack(
        [
            b_proj[h * 384 + half * 128 : h * 384 + half * 128 + 128]
            for h in range(NH)
            for half in range(2)
        ],
        axis=1,
    )  # [128, 2*NH], col t matches qk tile order
    b_v = WV_SCALE * np.concatenate(
        [b_proj[h * 384 + 256 : h * 384 + 384] for h in range(NH)]
    )  # [F]
    b_v_bcast = np.broadcast_to(np.concatenate([b_v, b_v]), (128, 2 * F))
    b_out_t = np.asarray(b_out, dtype=np.float32).reshape(KT, 128).T  # [128, KT]
    esh = np.full((128, 1), ESHIFT, dtype=np.float32)
    bias = np.ascontiguousarray(
        np.concatenate([b_qk, b_v_bcast, b_out_t, esh], axis=1), dtype=np.float32
    )  # [128, 2*NH + 2*F + KT + 1]
    return x_f, np.ascontiguousarray(w_qkT), np.ascontiguousarray(w_vT), w_outT, bias


def kernel(x, w_proj, b_proj, w_out, b_out, n_heads):
    from concourse.bass_utils import run_bass_kernel_spmd

    assert int(n_heads) == NH
    x_f, w_qkT, w_vT, w_outT, bias = _prep_inputs(x, w_proj, b_proj, w_out, b_out)

    if "nc" not in _CACHE:
        _CACHE["nc"] = _build()
    nc = _CACHE["nc"]

    in_maps = [
        {
            "x": np.ascontiguousarray(x_f[c * BL : (c + 1) * BL]),
            "w_qkT": w_qkT,
            "w_vT": w_vT,
            "w_outT": w_outT,
            "bias": bias,
        }
        for c in range(NCORES)
    ]
    res = run_bass_kernel_spmd(nc, in_maps, list(range(NCORES)))
    out = np.concatenate([res.results[c]["out"] for c in range(NCORES)], axis=0)
    return out.reshape(B, C, 32, 32)
